# revision 16
# baseline (speedup 1.0000x reference)
"""Trainium2 Bass kernel for blended-expert MLP (moe_routing).

Model: for each of 3 layers, w_l = sum_e c_e * W[l,e]  (E=8 experts),
x = act(x @ w_l.T + B_l), act = ELU for layers 0,1, none for layer 2.

Strategy (8 NeuronCores):
- Data-parallel over the 4096-row batch (512 rows/core).
- The expert blend is sharded over the contraction (input) dim: core k blends
  i-slice k (256 rows of w_l.T) on the Vector engine, then an 8-core
  AllGather assembles the full transposed blended weight w_l.T (2048x2048)
  in DRAM, which the matmul phase streams as stationary operands.
- Matmuls run in bf16 (fp32 PSUM accumulation; rel err ~4e-3 — set
  USE_BF16=False for float32r/TF32-like at ~2.6e-4 and ~30% more time).
  Activations stay SBUF-resident between layers in [feature, batch]
  orientation; ELU is fused DVE/ACT ops: max(exp(min(z,0))-1, z).
- Host side only reshapes/transposes/slices for sharding and assembles the
  output; all FLOPs (blend, matmul, bias, ELU) run on device.
"""

import numpy as np

import concourse.mybir as mybir
import concourse.tile as tile
from concourse import bacc
from concourse.bass_utils import run_bass_kernel_spmd

N_CORES = 8
L = 3          # layers
E = 8          # experts
D = 2048       # feature dim
BATCH = 4096
BS = BATCH // N_CORES   # 512 batch rows per core
IS = D // N_CORES       # 256 contraction rows blended per core
KT = D // 128           # 16 k-tiles
MT = D // 128           # 16 m-tiles (output feature tiles)
HALF_M = MT // 2        # 8 psum banks per half

f32 = mybir.dt.float32
f32r = mybir.dt.float32r
bf16 = mybir.dt.bfloat16

# When True, the blended weights travel through the AllGather and the weight
# stream in bf16 (half the bytes, ~15% faster matmuls) and activations are
# bf16 too; rel err ~4e-3 vs ~2.6e-4 for fp32r.
USE_BF16 = True

_cache: dict = {}


def _build():
    nc = bacc.Bacc("TRN2", target_bir_lowering=False, debug=False,
                   num_devices=N_CORES)
    indt = bf16 if USE_BF16 else f32
    # Per-core inputs (pre-sharded/transposed by host):
    # WtT: (L, E, IS, D) = this core's i-slice of W transposed to [in, out];
    # bf16 when USE_BF16 (the blend result is bf16-quantized anyway, so
    # quantizing the expert inputs costs ~1e-3 extra rel err and halves
    # the dominant HBM stream).
    WtT = nc.dram_tensor("WtT", [L, E, IS, D], indt, kind="ExternalInput")
    # xT: (D, BS) = this core's batch shard, transposed
    xT = nc.dram_tensor("xT", [D, BS], indt, kind="ExternalInput")
    # cb: (128, E) = weight_blend broadcast across partitions
    cb = nc.dram_tensor("cb", [128, E], f32, kind="ExternalInput")
    # Bp: (128, L*MT) = biases; col l*MT+m holds B[l, m*128 : (m+1)*128]
    Bp = nc.dram_tensor("Bp", [128, L * MT], f32, kind="ExternalInput")
    # Output: yT (D, BS) = this core's output shard, transposed
    yT = nc.dram_tensor("yT", [D, BS], f32, kind="ExternalOutput")

    with tile.TileContext(nc) as tc:
        with (
            tc.tile_pool(name="const", bufs=1) as cpool,
            tc.tile_pool(name="acts", bufs=1) as apool,
            tc.tile_pool(name="blend", bufs=1) as bpool,
            tc.tile_pool(name="exp", bufs=8) as epool,
            tc.tile_pool(name="wst", bufs=12) as wpool,
            tc.tile_pool(name="epi", bufs=4) as xpool,
            tc.tile_pool(name="psum", bufs=1, space="PSUM") as ppool,
            tc.tile_pool(name="dram", bufs=1, space="DRAM") as dram,
        ):
            cbt = cpool.tile([128, E], f32)
            bt = cpool.tile([128, L * MT], f32)
            nc.sync.dma_start(cbt[:], cb[:])
            nc.sync.dma_start(bt[:], Bp[:])

            # Warmup AllGather: absorbs cross-core launch skew and the
            # first-collective staging cost while the bulk loads run.
            wuin = dram.tile([1, E], f32, name="wuin")
            wuout = dram.tile([N_CORES, E], f32, addr_space="Shared",
                              name="wuout")
            nc.sync.dma_start(wuin[:], cb[:1, :])
            nc.gpsimd.collective_compute(
                "AllGather", mybir.AluOpType.bypass,
                ins=[wuin.opt()], outs=[wuout.opt()],
                replica_groups=[list(range(N_CORES))],
            )

            mmdt = bf16 if USE_BF16 else f32r
            # Activations: two sets of KT tiles [128, BS], ping-pong.
            acts = [
                [apool.tile([128, BS], mmdt, name=f"act{s}_{k}")
                 for k in range(KT)]
                for s in range(2)
            ]
            # DRAM bounce buffers for the weight AllGathers.
            agins = [dram.tile([IS, D], mmdt, name=f"agin{l}") for l in range(L)]
            agouts = [
                dram.tile([D, D], mmdt, addr_space="Shared", name=f"agout{l}")
                for l in range(L)
            ]

            # ---- blend + AllGather emission, one layer at a time.
            # Layers 0/1 go up-front; layer 2's blend is emitted between
            # mm0 and mm1 so its expert-load DMA traffic stays out of
            # AllGather-1's window (AG2 still completes with slack). ----
            def emit_blend(l):
                for h in range(IS // 128):  # 2 half-slices of 128 partitions
                    acc = bpool.tile([128, D], f32, name=f"acc{l}_{h}",
                                     tag=f"acc{h}", bufs=2)
                    accq = bpool.tile([128, D], mmdt, name=f"accq{l}_{h}",
                                      tag=f"accq{h}", bufs=2)
                    for e in range(E):
                        et = epool.tile([128, D], indt, name=f"exp{l}_{h}_{e}",
                                        tag="exp")
                        nc.scalar.dma_start(
                            et[:], WtT[l, e, h * 128:(h + 1) * 128, :]
                        )
                        if e == 0:
                            nc.vector.tensor_scalar_mul(
                                acc[:], et[:], cbt[:, 0:1]
                            )
                        elif e < E - 1:
                            # acc = (et * c_e) + acc
                            nc.vector.scalar_tensor_tensor(
                                acc[:], et[:], cbt[:, e:e + 1], acc[:],
                                mybir.AluOpType.mult, mybir.AluOpType.add,
                            )
                        else:
                            # last expert writes the matmul-dtype copy
                            nc.vector.scalar_tensor_tensor(
                                accq[:], et[:], cbt[:, e:e + 1], acc[:],
                                mybir.AluOpType.mult, mybir.AluOpType.add,
                            )
                    nc.gpsimd.dma_start(
                        agins[l][h * 128:(h + 1) * 128, :], accq[:]
                    )
                nc.gpsimd.collective_compute(
                    "AllGather", mybir.AluOpType.bypass,
                    ins=[agins[l].opt()], outs=[agouts[l].opt()],
                    replica_groups=[list(range(N_CORES))],
                )

            emit_blend(0)
            emit_blend(1)
            emit_blend(2)

            for k in range(KT):
                xsrc = xT[k * 128:(k + 1) * 128, :]
                nc.sync.dma_start(
                    acts[0][k][:], xsrc if USE_BF16 else xsrc.bitcast(f32r)
                )

            for l in range(L):
                # ---- matmul: y_l.T[m,:] = sum_k w_l.T[k,m].T @ act[k] ----
                src = acts[l % 2]
                dst = acts[(l + 1) % 2]
                for half in range(2):
                    psums = [
                        ppool.tile([128, BS], f32,
                                   name=f"ps{l}_{half}_{m}", tag=f"bank{m}")
                        for m in range(HALF_M)
                    ]
                    for k in range(KT):
                        ws = wpool.tile([128, HALF_M * 128], mmdt,
                                        name=f"ws{l}_{half}_{k}", tag="ws")
                        nc.sync.dma_start(
                            ws[:],
                            agouts[l][
                                k * 128:(k + 1) * 128,
                                half * HALF_M * 128:(half + 1) * HALF_M * 128,
                            ],
                        )
                        for m in range(HALF_M):
                            nc.tensor.matmul(
                                psums[m][:],
                                ws[:, m * 128:(m + 1) * 128],
                                src[k][:],
                                start=(k == 0),
                                stop=(k == KT - 1),
                            )
                    # ---- epilogue: bias (+ ELU), write next-layer acts ----
                    for m in range(HALF_M):
                        gm = half * HALF_M + m
                        bias = bt[:, l * MT + gm: l * MT + gm + 1]
                        ps = psums[m]
                        if l < L - 1:
                            tt = xpool.tile([128, BS], f32,
                                            name=f"t{l}_{gm}", tag="tmin")
                            zt = xpool.tile([128, BS], f32,
                                            name=f"z{l}_{gm}", tag="zbias")
                            ut = xpool.tile([128, BS], f32,
                                            name=f"u{l}_{gm}", tag="uexp")
                            # t = min(psum + bias, 0) on DVE
                            nc.vector.tensor_scalar(
                                tt[:], ps[:], bias, 0.0,
                                mybir.AluOpType.add, mybir.AluOpType.min,
                            )
                            # z = psum + bias on ACT
                            nc.scalar.activation(
                                zt[:], ps[:],
                                mybir.ActivationFunctionType.Identity,
                                bias=bias,
                            )
                            # u = exp(t) on ACT
                            nc.scalar.activation(
                                ut[:], tt[:], mybir.ActivationFunctionType.Exp
                            )
                            # act_next = max(u - 1, z) on DVE, in matmul dtype
                            nc.vector.scalar_tensor_tensor(
                                dst[gm][:], ut[:], 1.0, zt[:],
                                mybir.AluOpType.subtract, mybir.AluOpType.max,
                            )
                        else:
                            ot = xpool.tile([128, BS], f32,
                                            name=f"o{gm}", tag="outt")
                            nc.scalar.activation(
                                ot[:], ps[:],
                                mybir.ActivationFunctionType.Identity,
                                bias=bias,
                            )
                            nc.gpsimd.dma_start(
                                yT[gm * 128:(gm + 1) * 128, :], ot[:]
                            )
    nc.finalize()
    return nc


def _get_nc():
    if "nc" not in _cache:
        _cache["nc"] = _build()
    return _cache["nc"]


def kernel(weight_blend, x, W, B) -> np.ndarray:
    weight_blend = np.asarray(weight_blend, dtype=np.float32)
    x = np.asarray(x, dtype=np.float32)
    W = np.asarray(W, dtype=np.float32)
    B = np.asarray(B, dtype=np.float32)

    cb = np.ascontiguousarray(np.broadcast_to(weight_blend[None, :], (128, E)))
    # Bp[p, l*MT+m] = B[l, m*128+p]
    Bp = np.ascontiguousarray(
        B.reshape(L, MT, 128).transpose(2, 0, 1).reshape(128, L * MT)
    )

    import ml_dtypes
    indt = ml_dtypes.bfloat16 if USE_BF16 else np.float32
    in_maps = []
    for k in range(N_CORES):
        WtT = np.ascontiguousarray(
            W[:, :, :, k * IS:(k + 1) * IS].transpose(0, 1, 3, 2)
        ).astype(indt)
        xTk = np.ascontiguousarray(x[k * BS:(k + 1) * BS, :].T).astype(indt)
        in_maps.append({"WtT": WtT, "xT": xTk, "cb": cb, "Bp": Bp})

    nc = _get_nc()
    last_err = None
    for attempt in range(3):
        try:
            res = run_bass_kernel_spmd(nc, in_maps,
                                       core_ids=list(range(N_CORES)))
            break
        except Exception as e:  # transient NRT/device wedge: retry
            last_err = e
            import time as _time
            _time.sleep(10 * (attempt + 1))
    else:
        raise last_err

    out = np.empty((BATCH, D), dtype=np.float32)
    for k in range(N_CORES):
        out[k * BS:(k + 1) * BS, :] = res.results[k]["yT"].T
    return out


# revision 17
# speedup vs baseline: 1.0337x; 1.0337x over previous
"""Trainium2 Bass kernel for blended-expert MLP (moe_routing).

Model: for each of 3 layers, w_l = sum_e c_e * W[l,e]  (E=8 experts),
x = act(x @ w_l.T + B_l), act = ELU for layers 0,1, none for layer 2.

Strategy (8 NeuronCores):
- Data-parallel over the 4096-row batch (512 rows/core).
- The expert blend is sharded over the contraction (input) dim: core k blends
  i-slice k (256 rows of w_l.T) on the Vector engine, then an 8-core
  AllGather assembles the full transposed blended weight w_l.T (2048x2048)
  in DRAM, which the matmul phase streams as stationary operands.
- Matmuls run in bf16 (fp32 PSUM accumulation; rel err ~4e-3 — set
  USE_BF16=False for float32r/TF32-like at ~2.6e-4 and ~30% more time).
  Activations stay SBUF-resident between layers in [feature, batch]
  orientation; ELU is fused DVE/ACT ops: max(exp(min(z,0))-1, z).
- Host side only reshapes/transposes/slices for sharding and assembles the
  output; all FLOPs (blend, matmul, bias, ELU) run on device.
"""

import numpy as np

import concourse.mybir as mybir
import concourse.tile as tile
from concourse import bacc
from concourse.bass_utils import run_bass_kernel_spmd

N_CORES = 8
L = 3          # layers
E = 8          # experts
D = 2048       # feature dim
BATCH = 4096
BS = BATCH // N_CORES   # 512 batch rows per core
IS = D // N_CORES       # 256 contraction rows blended per core
KT = D // 128           # 16 k-tiles
MT = D // 128           # 16 m-tiles (output feature tiles)
HALF_M = MT // 2        # 8 psum banks per half

f32 = mybir.dt.float32
f32r = mybir.dt.float32r
bf16 = mybir.dt.bfloat16

# When True, the blended weights travel through the AllGather and the weight
# stream in bf16 (half the bytes, ~15% faster matmuls) and activations are
# bf16 too; rel err ~4e-3 vs ~2.6e-4 for fp32r.
USE_BF16 = True

_cache: dict = {}


def _build():
    nc = bacc.Bacc("TRN2", target_bir_lowering=False, debug=False,
                   num_devices=N_CORES)
    indt = bf16 if USE_BF16 else f32
    # Per-core inputs (pre-sharded/transposed by host):
    # WtT: (L, E, IS, D) = this core's i-slice of W transposed to [in, out];
    # bf16 when USE_BF16 (the blend result is bf16-quantized anyway, so
    # quantizing the expert inputs costs ~1e-3 extra rel err and halves
    # the dominant HBM stream).
    WtT = nc.dram_tensor("WtT", [L, E, IS, D], indt, kind="ExternalInput")
    # xT: (D, BS) = this core's batch shard, transposed
    xT = nc.dram_tensor("xT", [D, BS], indt, kind="ExternalInput")
    # cb: (128, E) = weight_blend broadcast across partitions
    cb = nc.dram_tensor("cb", [128, E], f32, kind="ExternalInput")
    # Bp: (128, L*MT) = biases; col l*MT+m holds B[l, m*128 : (m+1)*128]
    Bp = nc.dram_tensor("Bp", [128, L * MT], f32, kind="ExternalInput")
    # Output: yT (D, BS) = this core's output shard, transposed
    yT = nc.dram_tensor("yT", [D, BS], f32, kind="ExternalOutput")

    with tile.TileContext(nc) as tc:
        with (
            tc.tile_pool(name="const", bufs=1) as cpool,
            tc.tile_pool(name="acts", bufs=1) as apool,
            tc.tile_pool(name="blend", bufs=1) as bpool,
            tc.tile_pool(name="exp", bufs=8) as epool,
            tc.tile_pool(name="wst", bufs=12) as wpool,
            tc.tile_pool(name="epi", bufs=4) as xpool,
            tc.tile_pool(name="psum", bufs=1, space="PSUM") as ppool,
            tc.tile_pool(name="dram", bufs=1, space="DRAM") as dram,
        ):
            cbt = cpool.tile([128, E], f32)
            bt = cpool.tile([128, L * MT], f32)
            nc.sync.dma_start(cbt[:], cb[:])
            nc.sync.dma_start(bt[:], Bp[:])

            # Warmup AllGather: absorbs cross-core launch skew and the
            # first-collective staging cost while the bulk loads run.
            wuin = dram.tile([1, E], f32, name="wuin")
            wuout = dram.tile([N_CORES, E], f32, addr_space="Shared",
                              name="wuout")
            nc.sync.dma_start(wuin[:], cb[:1, :])
            nc.gpsimd.collective_compute(
                "AllGather", mybir.AluOpType.bypass,
                ins=[wuin.opt()], outs=[wuout.opt()],
                replica_groups=[list(range(N_CORES))],
            )

            mmdt = bf16 if USE_BF16 else f32r
            # Activations: two sets of KT tiles [128, BS], ping-pong.
            acts = [
                [apool.tile([128, BS], mmdt, name=f"act{s}_{k}")
                 for k in range(KT)]
                for s in range(2)
            ]
            # DRAM bounce buffers for the weight AllGathers.
            agins = [dram.tile([IS, D], mmdt, name=f"agin{l}") for l in range(L)]
            agouts = [
                dram.tile([D, D], mmdt, addr_space="Shared", name=f"agout{l}")
                for l in range(L)
            ]

            # ---- blend + AllGather emission, one layer at a time.
            # Layers 0/1 go up-front; layer 2's blend is emitted between
            # mm0 and mm1 so its expert-load DMA traffic stays out of
            # AllGather-1's window (AG2 still completes with slack). ----
            def emit_blend(l):
                for h in range(IS // 128):  # 2 half-slices of 128 partitions
                    acc = bpool.tile([128, D], f32, name=f"acc{l}_{h}",
                                     tag=f"acc{h}", bufs=2)
                    accq = bpool.tile([128, D], mmdt, name=f"accq{l}_{h}",
                                      tag=f"accq{h}", bufs=2)
                    for e in range(E):
                        et = epool.tile([128, D], indt, name=f"exp{l}_{h}_{e}",
                                        tag="exp")
                        nc.scalar.dma_start(
                            et[:], WtT[l, e, h * 128:(h + 1) * 128, :]
                        )
                        if e == 0:
                            nc.vector.tensor_scalar_mul(
                                acc[:], et[:], cbt[:, 0:1]
                            )
                        elif e < E - 1:
                            # acc = (et * c_e) + acc
                            nc.vector.scalar_tensor_tensor(
                                acc[:], et[:], cbt[:, e:e + 1], acc[:],
                                mybir.AluOpType.mult, mybir.AluOpType.add,
                            )
                        else:
                            # last expert writes the matmul-dtype copy
                            nc.vector.scalar_tensor_tensor(
                                accq[:], et[:], cbt[:, e:e + 1], acc[:],
                                mybir.AluOpType.mult, mybir.AluOpType.add,
                            )
                    nc.gpsimd.dma_start(
                        agins[l][h * 128:(h + 1) * 128, :], accq[:]
                    )
                nc.gpsimd.collective_compute(
                    "AllGather", mybir.AluOpType.bypass,
                    ins=[agins[l].opt()], outs=[agouts[l].opt()],
                    replica_groups=[list(range(N_CORES))],
                )

            emit_blend(0)
            emit_blend(1)

            for k in range(KT):
                xsrc = xT[k * 128:(k + 1) * 128, :]
                nc.sync.dma_start(
                    acts[0][k][:], xsrc if USE_BF16 else xsrc.bitcast(f32r)
                )

            for l in range(L):
                if l == 1:
                    emit_blend(2)
                # ---- matmul: y_l.T[m,:] = sum_k w_l.T[k,m].T @ act[k] ----
                src = acts[l % 2]
                dst = acts[(l + 1) % 2]
                for half in range(2):
                    psums = [
                        ppool.tile([128, BS], f32,
                                   name=f"ps{l}_{half}_{m}", tag=f"bank{m}")
                        for m in range(HALF_M)
                    ]
                    for k in range(KT):
                        ws = wpool.tile([128, HALF_M * 128], mmdt,
                                        name=f"ws{l}_{half}_{k}", tag="ws")
                        nc.sync.dma_start(
                            ws[:],
                            agouts[l][
                                k * 128:(k + 1) * 128,
                                half * HALF_M * 128:(half + 1) * HALF_M * 128,
                            ],
                        )
                        for m in range(HALF_M):
                            nc.tensor.matmul(
                                psums[m][:],
                                ws[:, m * 128:(m + 1) * 128],
                                src[k][:],
                                start=(k == 0),
                                stop=(k == KT - 1),
                            )
                    # ---- epilogue: bias (+ ELU), write next-layer acts ----
                    for m in range(HALF_M):
                        gm = half * HALF_M + m
                        bias = bt[:, l * MT + gm: l * MT + gm + 1]
                        ps = psums[m]
                        if l < L - 1:
                            tt = xpool.tile([128, BS], f32,
                                            name=f"t{l}_{gm}", tag="tmin")
                            zt = xpool.tile([128, BS], f32,
                                            name=f"z{l}_{gm}", tag="zbias")
                            ut = xpool.tile([128, BS], f32,
                                            name=f"u{l}_{gm}", tag="uexp")
                            # t = min(psum + bias, 0) on DVE
                            nc.vector.tensor_scalar(
                                tt[:], ps[:], bias, 0.0,
                                mybir.AluOpType.add, mybir.AluOpType.min,
                            )
                            # z = psum + bias on ACT
                            nc.scalar.activation(
                                zt[:], ps[:],
                                mybir.ActivationFunctionType.Identity,
                                bias=bias,
                            )
                            # u = exp(t) on ACT
                            nc.scalar.activation(
                                ut[:], tt[:], mybir.ActivationFunctionType.Exp
                            )
                            # act_next = max(u - 1, z) on DVE, in matmul dtype
                            nc.vector.scalar_tensor_tensor(
                                dst[gm][:], ut[:], 1.0, zt[:],
                                mybir.AluOpType.subtract, mybir.AluOpType.max,
                            )
                        else:
                            ot = xpool.tile([128, BS], f32,
                                            name=f"o{gm}", tag="outt")
                            nc.scalar.activation(
                                ot[:], ps[:],
                                mybir.ActivationFunctionType.Identity,
                                bias=bias,
                            )
                            nc.gpsimd.dma_start(
                                yT[gm * 128:(gm + 1) * 128, :], ot[:]
                            )
    nc.finalize()
    return nc


def _get_nc():
    if "nc" not in _cache:
        _cache["nc"] = _build()
    return _cache["nc"]


def kernel(weight_blend, x, W, B) -> np.ndarray:
    weight_blend = np.asarray(weight_blend, dtype=np.float32)
    x = np.asarray(x, dtype=np.float32)
    W = np.asarray(W, dtype=np.float32)
    B = np.asarray(B, dtype=np.float32)

    cb = np.ascontiguousarray(np.broadcast_to(weight_blend[None, :], (128, E)))
    # Bp[p, l*MT+m] = B[l, m*128+p]
    Bp = np.ascontiguousarray(
        B.reshape(L, MT, 128).transpose(2, 0, 1).reshape(128, L * MT)
    )

    import ml_dtypes
    indt = ml_dtypes.bfloat16 if USE_BF16 else np.float32
    in_maps = []
    for k in range(N_CORES):
        WtT = np.ascontiguousarray(
            W[:, :, :, k * IS:(k + 1) * IS].transpose(0, 1, 3, 2)
        ).astype(indt)
        xTk = np.ascontiguousarray(x[k * BS:(k + 1) * BS, :].T).astype(indt)
        in_maps.append({"WtT": WtT, "xT": xTk, "cb": cb, "Bp": Bp})

    nc = _get_nc()
    last_err = None
    for attempt in range(3):
        try:
            res = run_bass_kernel_spmd(nc, in_maps,
                                       core_ids=list(range(N_CORES)))
            break
        except Exception as e:  # transient NRT/device wedge: retry
            last_err = e
            import time as _time
            _time.sleep(10 * (attempt + 1))
    else:
        raise last_err

    out = np.empty((BATCH, D), dtype=np.float32)
    for k in range(N_CORES):
        out[k * BS:(k + 1) * BS, :] = res.results[k]["yT"].T
    return out


# revision 18
# speedup vs baseline: 1.0414x; 1.0075x over previous
"""Trainium2 Bass kernel for blended-expert MLP (moe_routing).

Model: for each of 3 layers, w_l = sum_e c_e * W[l,e]  (E=8 experts),
x = act(x @ w_l.T + B_l), act = ELU for layers 0,1, none for layer 2.

Strategy (8 NeuronCores):
- Data-parallel over the 4096-row batch (512 rows/core).
- The expert blend is sharded over the contraction (input) dim: core k blends
  i-slice k (256 rows of w_l.T) on the Vector engine, then an 8-core
  AllGather assembles the full transposed blended weight w_l.T (2048x2048)
  in DRAM, which the matmul phase streams as stationary operands.
- Matmuls run in bf16 (fp32 PSUM accumulation; rel err ~4e-3 — set
  USE_BF16=False for float32r/TF32-like at ~2.6e-4 and ~30% more time).
  Activations stay SBUF-resident between layers in [feature, batch]
  orientation; ELU is fused DVE/ACT ops: max(exp(min(z,0))-1, z).
- Host side only reshapes/transposes/slices for sharding and assembles the
  output; all FLOPs (blend, matmul, bias, ELU) run on device.
"""

import numpy as np

import concourse.mybir as mybir
import concourse.tile as tile
from concourse import bacc
from concourse.bass_utils import run_bass_kernel_spmd

N_CORES = 8
L = 3          # layers
E = 8          # experts
D = 2048       # feature dim
BATCH = 4096
BS = BATCH // N_CORES   # 512 batch rows per core
IS = D // N_CORES       # 256 contraction rows blended per core
KT = D // 128           # 16 k-tiles
MT = D // 128           # 16 m-tiles (output feature tiles)
HALF_M = MT // 2        # 8 psum banks per half

f32 = mybir.dt.float32
f32r = mybir.dt.float32r
bf16 = mybir.dt.bfloat16

# When True, the blended weights travel through the AllGather and the weight
# stream in bf16 (half the bytes, ~15% faster matmuls) and activations are
# bf16 too; rel err ~4e-3 vs ~2.6e-4 for fp32r.
USE_BF16 = True

_cache: dict = {}


def _build():
    nc = bacc.Bacc("TRN2", target_bir_lowering=False, debug=False,
                   num_devices=N_CORES)
    indt = bf16 if USE_BF16 else f32
    # Per-core inputs (pre-sharded/transposed by host):
    # WtT: (L, E, IS, D) = this core's i-slice of W transposed to [in, out];
    # bf16 when USE_BF16 (the blend result is bf16-quantized anyway, so
    # quantizing the expert inputs costs ~1e-3 extra rel err and halves
    # the dominant HBM stream).
    WtT = nc.dram_tensor("WtT", [L, E, IS, D], indt, kind="ExternalInput")
    # xT: (D, BS) = this core's batch shard, transposed
    xT = nc.dram_tensor("xT", [D, BS], indt, kind="ExternalInput")
    # cb: (128, E) = weight_blend broadcast across partitions
    cb = nc.dram_tensor("cb", [128, E], f32, kind="ExternalInput")
    # Bp: (128, L*MT) = biases; col l*MT+m holds B[l, m*128 : (m+1)*128]
    Bp = nc.dram_tensor("Bp", [128, L * MT], f32, kind="ExternalInput")
    # Output: yT (D, BS) = this core's output shard, transposed
    yT = nc.dram_tensor("yT", [D, BS], f32, kind="ExternalOutput")

    with tile.TileContext(nc) as tc:
        with (
            tc.tile_pool(name="const", bufs=1) as cpool,
            tc.tile_pool(name="acts", bufs=1) as apool,
            tc.tile_pool(name="blend", bufs=1) as bpool,
            tc.tile_pool(name="exp", bufs=8) as epool,
            tc.tile_pool(name="wst", bufs=12) as wpool,
            tc.tile_pool(name="epi", bufs=4) as xpool,
            tc.tile_pool(name="psum", bufs=1, space="PSUM") as ppool,
            tc.tile_pool(name="dram", bufs=1, space="DRAM") as dram,
        ):
            cbt = cpool.tile([128, E], f32)
            bt = cpool.tile([128, L * MT], f32)
            nc.sync.dma_start(cbt[:], cb[:])
            nc.sync.dma_start(bt[:], Bp[:])

            # Warmup AllGather: absorbs cross-core launch skew and the
            # first-collective staging cost while the bulk loads run.
            wuin = dram.tile([1, E], f32, name="wuin")
            wuout = dram.tile([N_CORES, E], f32, addr_space="Shared",
                              name="wuout")
            nc.sync.dma_start(wuin[:], cb[:1, :])
            nc.gpsimd.collective_compute(
                "AllGather", mybir.AluOpType.bypass,
                ins=[wuin.opt()], outs=[wuout.opt()],
                replica_groups=[list(range(N_CORES))],
            )

            mmdt = bf16 if USE_BF16 else f32r
            # Activations: two sets of KT tiles [128, BS], ping-pong.
            acts = [
                [apool.tile([128, BS], mmdt, name=f"act{s}_{k}")
                 for k in range(KT)]
                for s in range(2)
            ]
            # DRAM bounce buffers for the weight AllGathers.
            agins = [dram.tile([IS, D], mmdt, name=f"agin{l}") for l in range(L)]
            agouts = [
                dram.tile([D, D], mmdt, addr_space="Shared", name=f"agout{l}")
                for l in range(L)
            ]

            # ---- blend + AllGather emission, one layer at a time.
            # Layers 0/1 go up-front; layer 2's blend is emitted between
            # mm0 and mm1 so its expert-load DMA traffic stays out of
            # AllGather-1's window (AG2 still completes with slack). ----
            def emit_blend(l):
                for h in range(IS // 128):  # 2 half-slices of 128 partitions
                    acc = bpool.tile([128, D], f32, name=f"acc{l}_{h}",
                                     tag=f"acc{h}", bufs=2)
                    accq = bpool.tile([128, D], mmdt, name=f"accq{l}_{h}",
                                      tag=f"accq{h}", bufs=2)
                    for e in range(E):
                        et = epool.tile([128, D], indt, name=f"exp{l}_{h}_{e}",
                                        tag="exp")
                        nc.scalar.dma_start(
                            et[:], WtT[l, e, h * 128:(h + 1) * 128, :]
                        )
                        if e == 0:
                            nc.vector.tensor_scalar_mul(
                                acc[:], et[:], cbt[:, 0:1]
                            )
                        elif e < E - 1:
                            # acc = (et * c_e) + acc
                            nc.vector.scalar_tensor_tensor(
                                acc[:], et[:], cbt[:, e:e + 1], acc[:],
                                mybir.AluOpType.mult, mybir.AluOpType.add,
                            )
                        else:
                            # last expert writes the matmul-dtype copy
                            nc.vector.scalar_tensor_tensor(
                                accq[:], et[:], cbt[:, e:e + 1], acc[:],
                                mybir.AluOpType.mult, mybir.AluOpType.add,
                            )
                    nc.gpsimd.dma_start(
                        agins[l][h * 128:(h + 1) * 128, :], accq[:]
                    )
                nc.gpsimd.collective_compute(
                    "AllGather", mybir.AluOpType.bypass,
                    ins=[agins[l].opt()], outs=[agouts[l].opt()],
                    replica_groups=[list(range(N_CORES))],
                )

            emit_blend(0)
            emit_blend(1)

            for k in range(KT):
                xsrc = xT[k * 128:(k + 1) * 128, :]
                nc.sync.dma_start(
                    acts[0][k][:], xsrc if USE_BF16 else xsrc.bitcast(f32r)
                )

            for l in range(L):
                if l == 1:
                    emit_blend(2)
                # ---- matmul: y_l.T[m,:] = sum_k w_l.T[k,m].T @ act[k] ----
                src = acts[l % 2]
                dst = acts[(l + 1) % 2]
                for half in range(2):
                    psums = [
                        ppool.tile([128, BS], f32,
                                   name=f"ps{l}_{half}_{m}", tag=f"bank{m}")
                        for m in range(HALF_M)
                    ]
                    for k in range(KT):
                        ws = wpool.tile([128, HALF_M * 128], mmdt,
                                        name=f"ws{l}_{half}_{k}", tag="ws")
                        nc.sync.dma_start(
                            ws[:],
                            agouts[l][
                                k * 128:(k + 1) * 128,
                                half * HALF_M * 128:(half + 1) * HALF_M * 128,
                            ],
                        )
                        for m in range(HALF_M):
                            nc.tensor.matmul(
                                psums[m][:],
                                ws[:, m * 128:(m + 1) * 128],
                                src[k][:],
                                start=(k == 0),
                                stop=(k == KT - 1),
                            )
                    # ---- epilogue: bias (+ ELU), write next-layer acts ----
                    for m in range(HALF_M):
                        gm = half * HALF_M + m
                        bias = bt[:, l * MT + gm: l * MT + gm + 1]
                        ps = psums[m]
                        if l < L - 1:
                            tt = xpool.tile([128, BS], f32,
                                            name=f"t{l}_{gm}", tag="tmin")
                            zt = xpool.tile([128, BS], f32,
                                            name=f"z{l}_{gm}", tag="zbias")
                            ut = xpool.tile([128, BS], f32,
                                            name=f"u{l}_{gm}", tag="uexp")
                            # t = min(psum + bias, 0) on DVE
                            nc.vector.tensor_scalar(
                                tt[:], ps[:], bias, 0.0,
                                mybir.AluOpType.add, mybir.AluOpType.min,
                            )
                            # z = psum + bias on ACT
                            nc.scalar.activation(
                                zt[:], ps[:],
                                mybir.ActivationFunctionType.Identity,
                                bias=bias,
                            )
                            # u = exp(t) on ACT
                            nc.scalar.activation(
                                ut[:], tt[:], mybir.ActivationFunctionType.Exp
                            )
                            # act_next = max(u - 1, z) on DVE, in matmul dtype
                            nc.vector.scalar_tensor_tensor(
                                dst[gm][:], ut[:], 1.0, zt[:],
                                mybir.AluOpType.subtract, mybir.AluOpType.max,
                            )
                        else:
                            ot = xpool.tile([128, BS], f32,
                                            name=f"o{gm}", tag="outt")
                            nc.scalar.activation(
                                ot[:], ps[:],
                                mybir.ActivationFunctionType.Identity,
                                bias=bias,
                            )
                            nc.gpsimd.dma_start(
                                yT[gm * 128:(gm + 1) * 128, :], ot[:]
                            )
    nc.finalize()
    return nc


def _get_nc():
    if "nc" not in _cache:
        _cache["nc"] = _build()
    return _cache["nc"]


def make_in_maps(weight_blend, x, W, B):
    weight_blend = np.asarray(weight_blend, dtype=np.float32)
    x = np.asarray(x, dtype=np.float32)
    W = np.asarray(W, dtype=np.float32)
    B = np.asarray(B, dtype=np.float32)

    cb = np.ascontiguousarray(np.broadcast_to(weight_blend[None, :], (128, E)))
    # Bp[p, l*MT+m] = B[l, m*128+p]
    Bp = np.ascontiguousarray(
        B.reshape(L, MT, 128).transpose(2, 0, 1).reshape(128, L * MT)
    )

    import ml_dtypes
    indt = ml_dtypes.bfloat16 if USE_BF16 else np.float32
    in_maps = []
    for k in range(N_CORES):
        WtT = np.ascontiguousarray(
            W[:, :, :, k * IS:(k + 1) * IS].transpose(0, 1, 3, 2)
        ).astype(indt)
        xTk = np.ascontiguousarray(x[k * BS:(k + 1) * BS, :].T).astype(indt)
        in_maps.append({"WtT": WtT, "xT": xTk, "cb": cb, "Bp": Bp})
    return in_maps


def kernel(weight_blend, x, W, B) -> np.ndarray:
    in_maps = make_in_maps(weight_blend, x, W, B)
    nc = _get_nc()
    last_err = None
    for attempt in range(3):
        try:
            res = run_bass_kernel_spmd(nc, in_maps,
                                       core_ids=list(range(N_CORES)))
            break
        except Exception as e:  # transient NRT/device wedge: retry
            last_err = e
            import time as _time
            _time.sleep(10 * (attempt + 1))
    else:
        raise last_err

    out = np.empty((BATCH, D), dtype=np.float32)
    for k in range(N_CORES):
        out[k * BS:(k + 1) * BS, :] = res.results[k]["yT"].T
    return out
